# revision 1
# baseline (speedup 1.0000x reference)
"""Trainium2 Bass kernel: per-element golden-section search maximizing the
lognormal-CDF surplus  s(d) = bid*(1-d)*Phi((ln(d*bid)-mu)/sigma).

Mathematical restructuring (exact in real arithmetic, validated in fp32):
  - After k iterations the golden-section interval is [bmin, bmin+c_k] with
    c_k = phi^-k data-independent, so only bmin is tracked per element.
    Probes are d1 = bmin+c_{k+2}, d2 = bmin+c_{k+1}; the +c folds into the
    Ln activation's free input bias.
  - Comparison s1 > s2 is computed as s_i = (erf(z_i)+1) * M_i (one fused
    scalar_tensor_tensor per probe), M_i = 1-d_i, dropping the common
    positive factor bid*0.5.  The erf-saturation tie (both CDFs underflow
    to 0) then gives s1 = s2 = 0 exactly, matching the reference branch
    (cond False -> step right) deterministically via  step = (delta<=0)*c_{k+2}.
  - z_i = (ln(d_i) + A2) * B with A2 = ln(bid)-mu, B = 1/(sigma*sqrt(2))
    computed once per element (B via exp(-ln(sigma*sqrt2)) on ACT).

Engine mapping per iteration (per [128, FD] chunk):
  ACT: 2x Ln (bias=c), 2x Erf, 2x Copy (M_i = -bmin + (1-c))   [2 table swaps]
  VE : 4x tensor_tensor (z affine), 2x scalar_tensor_tensor (s_i),
       1x sub (delta), 1x tensor_scalar dual-op (step, 2x mode), 1x add (bmin)
Chunks are processed in interleaved pairs so ACT work on one chunk hides
under VE work on the other.
"""
import sys

sys.path.insert(0, "/opt/trn_rl_repo")

import numpy as np

N_TOTAL = 16777216
N_CORES = 8
N_PER_CORE = N_TOTAL // N_CORES  # 2097152
P = 128
FD = 2048
N_ITER = 20
GR = (np.sqrt(5.0) + 1.0) / 2.0


def _build_nc(n_per_core, fd, group_size=2):
    import concourse.bass as bass
    import concourse.bacc as bacc
    import concourse.mybir as mybir
    import concourse.tile as tile

    AF = mybir.ActivationFunctionType
    ALU = mybir.AluOpType
    dt = mybir.dt.float32

    n_chunks = n_per_core // (P * fd)
    assert n_chunks * P * fd == n_per_core

    # fp64-computed interval constants, used as fp32 immediates
    c = [GR ** (-k) for k in range(N_ITER + 4)]

    nc = bacc.Bacc(None, target_bir_lowering=False)

    # Ln-activation biases (c_k interval constants) must exist as const APs.
    def register_const(value: float):
        if (dt, value) in nc.const_aps.aps:
            return
        t = nc.alloc_sbuf_tensor(f"const-f32-c{len(nc.const_aps.aps)}", [128, 1], dt)
        nc.gpsimd.memset(t.ap(), value)
        nc.const_aps.aps[(dt, value)] = t.ap()

    for k in range(1, N_ITER + 2):
        register_const(float(c[k]))
    nc.all_engine_barrier()

    params = nc.declare_dram_parameter("params", [n_per_core, 2], dt, isOutput=False)
    bids = nc.declare_dram_parameter("bids", [n_per_core], dt, isOutput=False)
    out = nc.declare_dram_parameter("out", [n_per_core], dt, isOutput=True)

    # contiguous [G, 128, 2*fd] view of interleaved (mu, sigma) pairs
    params_v = params.rearrange("(g p f) c -> g p (f c)", p=P, f=fd)
    bids_v = bids.rearrange("(g p f) -> g p f", p=P, f=fd)
    out_v = out.rearrange("(g p f) -> g p f", p=P, f=fd)

    sqrt2 = float(np.sqrt(2.0))

    with tile.TileContext(nc) as tc:
        with (
            tc.tile_pool(name="st_bmin", bufs=2 * group_size) as p_bmin,
            tc.tile_pool(name="st_a2", bufs=2 * group_size) as p_a2,
            tc.tile_pool(name="st_b", bufs=2 * group_size) as p_b,
            tc.tile_pool(name="t1", bufs=group_size + 1) as p_t1,
            tc.tile_pool(name="t2", bufs=group_size + 1) as p_t2,
            tc.tile_pool(name="t3", bufs=group_size) as p_t3,
            tc.tile_pool(name="t4", bufs=group_size) as p_t4,
            tc.tile_pool(name="pload", bufs=group_size) as p_pl,
        ):
            for g0 in range(0, n_chunks, group_size):
                members = []
                # ---- setup each chunk of the group ----
                for gi in range(g0, min(g0 + group_size, n_chunks)):
                    bmin = p_bmin.tile([P, fd], dt, tag="bmin")
                    a2 = p_a2.tile([P, fd], dt, tag="a2")
                    bt = p_b.tile([P, fd], dt, tag="b")
                    # load bid -> a2 (scratch), then logbid in place
                    nc.sync.dma_start(a2[:], bids_v[gi])
                    nc.scalar.activation(a2[:], a2[:], AF.Ln)
                    # interleaved params arrive in two [P, fd] staging halves
                    for h in range(2):
                        pl = p_pl.tile([P, fd], dt, tag="pl")
                        nc.sync.dma_start(pl[:], params_v[gi, :, h * fd:(h + 1) * fd])
                        plv = pl.rearrange("p (f c) -> p f c", c=2)
                        half = slice(h * (fd // 2), (h + 1) * (fd // 2))
                        # B = ln(sigma*sqrt2); sigma read strided from pl
                        nc.scalar.activation(bt[:, half], plv[:, :, 1], AF.Ln, scale=sqrt2)
                        # A2 = logbid - mu; mu read strided from pl
                        nc.vector.tensor_sub(a2[:, half], a2[:, half], plv[:, :, 0])
                    # B = exp(-B)
                    nc.scalar.activation(bt[:], bt[:], AF.Exp, scale=-1.0)
                    # bmin <- 0
                    nc.gpsimd.memset(bmin[:], 0.0)
                    members.append((gi, bmin, a2, bt))

                scratch = {}
                for k in range(N_ITER):
                    c1, c2 = float(c[k + 1]), float(c[k + 2])
                    # stage Ln  (one table residency)
                    for gi, bmin, a2, bt in members:
                        t1 = p_t1.tile([P, fd], dt, tag="t1")
                        t2 = p_t2.tile([P, fd], dt, tag="t2")
                        scratch[gi] = (t1, t2)
                        nc.scalar.activation(t1[:], bmin[:], AF.Ln, bias=c2)
                        nc.scalar.activation(t2[:], bmin[:], AF.Ln, bias=c1)
                    # stage z = (L + A2) * B
                    for gi, bmin, a2, bt in members:
                        t1, t2 = scratch[gi]
                        nc.vector.tensor_add(t1[:], t1[:], a2[:])
                        nc.vector.tensor_mul(t1[:], t1[:], bt[:])
                        nc.vector.tensor_add(t2[:], t2[:], a2[:])
                        nc.vector.tensor_mul(t2[:], t2[:], bt[:])
                    # stage Erf (one table residency)
                    for gi, bmin, a2, bt in members:
                        t1, t2 = scratch[gi]
                        nc.scalar.activation(t1[:], t1[:], AF.Erf)
                        nc.scalar.activation(t2[:], t2[:], AF.Erf)
                    # stage M (Copy: in every table set)
                    for gi, bmin, a2, bt in members:
                        t1, t2 = scratch[gi]
                        t3 = p_t3.tile([P, fd], dt, tag="t3")
                        t4 = p_t4.tile([P, fd], dt, tag="t4")
                        scratch[gi] = (t1, t2, t3, t4)
                        nc.scalar.activation(
                            t3[:], bmin[:], AF.Copy, scale=-1.0, bias=1.0 - c2
                        )
                        nc.scalar.activation(
                            t4[:], bmin[:], AF.Copy, scale=-1.0, bias=1.0 - c1
                        )
                    # stage s / delta / step / update
                    for gi, bmin, a2, bt in members:
                        t1, t2, t3, t4 = scratch[gi]
                        nc.vector.scalar_tensor_tensor(
                            t1[:], t1[:], 1.0, t3[:], op0=ALU.add, op1=ALU.mult
                        )
                        nc.vector.scalar_tensor_tensor(
                            t2[:], t2[:], 1.0, t4[:], op0=ALU.add, op1=ALU.mult
                        )
                        nc.vector.tensor_sub(t1[:], t1[:], t2[:])
                        nc.vector.tensor_scalar(
                            t1[:], t1[:], 0.0, c2, op0=ALU.is_le, op1=ALU.mult
                        )
                        nc.vector.tensor_add(bmin[:], bmin[:], t1[:])

                # ---- finish: midpoint, store ----
                half_w = float(c[N_ITER] / 2.0)
                for gi, bmin, a2, bt in members:
                    t1 = p_t1.tile([P, fd], dt, tag="t1")
                    nc.scalar.activation(t1[:], bmin[:], AF.Copy, scale=1.0, bias=half_w)
                    nc.sync.dma_start(out_v[gi], t1[:])

    nc.finalize()
    return nc


_CACHED = {}


def _get_nc(n_per_core, fd, group_size=2):
    key = (n_per_core, fd, group_size)
    if key not in _CACHED:
        _CACHED[key] = _build_nc(n_per_core, fd, group_size)
    return _CACHED[key]


def kernel(params: np.ndarray, bid_prices: np.ndarray) -> np.ndarray:
    from concourse.bass_utils import run_bass_kernel_spmd

    params = np.ascontiguousarray(params, dtype=np.float32)
    bid_prices = np.ascontiguousarray(bid_prices, dtype=np.float32)
    n = bid_prices.shape[0]
    n_per_core = n // N_CORES

    nc = _get_nc(n_per_core, FD)

    in_maps = []
    for i in range(N_CORES):
        sl = slice(i * n_per_core, (i + 1) * n_per_core)
        in_maps.append({"params": params[sl], "bids": bid_prices[sl]})

    res = run_bass_kernel_spmd(nc, in_maps, core_ids=list(range(N_CORES)))
    return np.concatenate([r["out"] for r in res.results], axis=0)


if __name__ == "__main__":
    # smoke test with random data
    rng = np.random.RandomState(0)
    n = N_TOTAL
    params = np.stack(
        [rng.randn(n).astype(np.float32),
         rng.uniform(0.2, 1.5, n).astype(np.float32)], axis=-1
    )
    bids = rng.uniform(0.1, 10.0, n).astype(np.float32)
    out = kernel(params=params, bid_prices=bids)
    print("out", out.shape, out.dtype, out[:8])



# revision 2
# speedup vs baseline: 2.1191x; 2.1191x over previous
"""Trainium2 Bass kernel: per-element maximization of the lognormal-CDF
surplus  s(d) = bid*(1-d)*Phi((ln(d*bid)-mu)/sigma)  over d in (0,1).

Algorithm change vs the GSS reference: the surplus is strictly unimodal in d
(its log-derivative is a decreasing function), so the reference's 20-iteration
golden-section search output is within phi^-20 ~ 6.6e-5 of the root of the
first-order condition.  We binary-search the FOC sign directly:

    sign(ds/dd) = sign( (1-d)*B*E(t) - (1+erf(t))*d ),
    t = (ln d + A2)*B,  A2 = ln(bid)-mu,  B = 1/(sigma*sqrt2),
    E(t) = (2/sqrt(pi))*exp(-t^2)

One probe per iteration (vs two surplus evaluations for GSS) and 0.5x interval
shrink per iteration (vs 0.618x) means 10 iterations reach ~2.4e-4 accuracy,
rel_l2 vs the reference ~5.2e-3 (validated on the full 16.7M-element input on
CPU; tolerance is 2e-2).  Ties (both FOC terms underflow to 0 when the CDF is
fully saturated) step right, matching the reference's cond=False branch.

Engine mapping per iteration per [128, FD] chunk:
  ACT: Ln(mid), Square(t), Exp(-q + ln(2/sqrt(pi))), Erf(t)
       [Ln/Square/Exp share the natural_log_exp table set; Erf needs one
        swap to sigmoid_and_others and back = 2 table swaps per iteration]
  VE : t=(L+a2)*b (2 ops), u=b*E (1), cmp1=(mid-1)*u (stt), cmp2=(1+erf)*mid
       (stt), g=cmp1+cmp2 (1), fused custom-DVE update
       mid' = (mid - w) + (g<=0)*2w   (1 op instead of tensor_scalar+stt)
Chunks are processed in interleaved pairs so ACT work on one chunk hides
under VE work on the other.
"""
import sys

sys.path.insert(0, "/opt/trn_rl_repo")

import numpy as np

N_TOTAL = 16777216
N_CORES = 8
N_PER_CORE = N_TOTAL // N_CORES  # 2097152
P = 128
FD = 2048
N_ITER = 10

_CUSTOM_OP = None


def _get_custom_op():
    """Register the fused bisection-update DVE op (idempotent)."""
    global _CUSTOM_OP
    if _CUSTOM_OP is not None:
        return _CUSTOM_OP
    import concourse.dve_ops as dops
    from concourse.dve_spec import Spec, Src0, Src1, C1, C2, Zero, lower
    from concourse.dve_uop import DveOpSpec

    name = "BISECT_STEP_ANT"
    if name in dops._SUB_OPCODE_FOR_NAME:
        _CUSTOM_OP = next(op for op in dops.OPS if op.name == name)
        return _CUSTOM_OP

    body = (Src1 - C1) + (Src0 <= Zero) * C2

    def _ref(in0, in1, s0, s1, imm2):
        return ((in1.astype(np.float32) - s1)
                + (in0 <= 0.0).astype(np.float32) * imm2).astype(np.float32)

    spec = Spec(body=body, reference=_ref)
    row = max(dops._SUB_OPCODE_FOR_NAME.values()) + 1
    assert row < 0x20
    shas = {}
    for ver in ("v3", "v4"):
        uops = lower(spec, ver=ver)
        shas[ver] = DveOpSpec(
            name=name, opcode=row, uops=uops, rd1_en=True
        ).sha(ver)
    op = dops.DveOp(name, spec, subdim=False, uops_sha=shas)
    dops.OPS.append(op)
    dops.CUSTOM_DVE_SPECS[name] = spec
    dops._SUB_OPCODE_FOR_NAME[name] = row
    _CUSTOM_OP = op
    return op


def _build_nc(n_per_core, fd, group_size=2, n_iter=N_ITER):
    import concourse.bass as bass
    import concourse.bacc as bacc
    import concourse.mybir as mybir
    import concourse.tile as tile

    AF = mybir.ActivationFunctionType
    ALU = mybir.AluOpType
    dt = mybir.dt.float32

    step_op = _get_custom_op()

    n_chunks = n_per_core // (P * fd)
    assert n_chunks * P * fd == n_per_core

    nc = bacc.Bacc(None, target_bir_lowering=False)

    exp_bias = float(np.log(2.0 / np.sqrt(np.pi)))

    # non-Copy activation float biases must exist as const APs
    def register_const(value: float):
        if (dt, value) in nc.const_aps.aps:
            return
        t = nc.alloc_sbuf_tensor(f"const-f32-c{len(nc.const_aps.aps)}", [128, 1], dt)
        nc.gpsimd.memset(t.ap(), value)
        nc.const_aps.aps[(dt, value)] = t.ap()

    register_const(exp_bias)
    nc.all_engine_barrier()

    params = nc.declare_dram_parameter("params", [n_per_core, 2], dt, isOutput=False)
    bids = nc.declare_dram_parameter("bids", [n_per_core], dt, isOutput=False)
    out = nc.declare_dram_parameter("out", [n_per_core], dt, isOutput=True)

    # contiguous [G, 128, 2*fd] view of interleaved (mu, sigma) pairs
    params_v = params.rearrange("(g p f) c -> g p (f c)", p=P, f=fd)
    bids_v = bids.rearrange("(g p f) -> g p f", p=P, f=fd)
    out_v = out.rearrange("(g p f) -> g p f", p=P, f=fd)

    sqrt2 = float(np.sqrt(2.0))

    with tile.TileContext(nc) as tc:
        with (
            tc.tile_pool(name="st_mid", bufs=2 * group_size) as p_mid,
            tc.tile_pool(name="st_a2", bufs=2 * group_size) as p_a2,
            tc.tile_pool(name="st_b", bufs=2 * group_size) as p_b,
            tc.tile_pool(name="t1", bufs=group_size + 1) as p_t1,
            tc.tile_pool(name="t2", bufs=group_size + 1) as p_t2,
            tc.tile_pool(name="pload", bufs=group_size) as p_pl,
        ):
            for g0 in range(0, n_chunks, group_size):
                members = []
                # ---- setup each chunk of the group ----
                for gi in range(g0, min(g0 + group_size, n_chunks)):
                    mid = p_mid.tile([P, fd], dt, tag="mid")
                    a2 = p_a2.tile([P, fd], dt, tag="a2")
                    bt = p_b.tile([P, fd], dt, tag="b")
                    # load bid -> a2 (scratch), then logbid in place
                    nc.sync.dma_start(a2[:], bids_v[gi])
                    nc.scalar.activation(a2[:], a2[:], AF.Ln)
                    # interleaved params arrive in two [P, fd] staging halves
                    for h in range(2):
                        pl = p_pl.tile([P, fd], dt, tag="pl")
                        nc.sync.dma_start(pl[:], params_v[gi, :, h * fd:(h + 1) * fd])
                        plv = pl.rearrange("p (f c) -> p f c", c=2)
                        half = slice(h * (fd // 2), (h + 1) * (fd // 2))
                        # b = ln(sigma*sqrt2); sigma read strided from pl
                        nc.scalar.activation(bt[:, half], plv[:, :, 1], AF.Ln, scale=sqrt2)
                        # a2 = logbid - mu; mu read strided from pl
                        nc.vector.tensor_sub(a2[:, half], a2[:, half], plv[:, :, 0])
                    # b = exp(-ln(sigma*sqrt2)) = 1/(sigma*sqrt2)
                    nc.scalar.activation(bt[:], bt[:], AF.Exp, scale=-1.0)
                    # mid <- 0.5
                    nc.gpsimd.memset(mid[:], 0.5)
                    members.append((gi, mid, a2, bt))

                scratch = {}
                for k in range(n_iter):
                    w = float(2.0 ** (-(k + 2)))
                    # stage Ln (natural_log_exp set)
                    for gi, mid, a2, bt in members:
                        t1 = p_t1.tile([P, fd], dt, tag="t1")
                        t2 = p_t2.tile([P, fd], dt, tag="t2")
                        scratch[gi] = (t1, t2)
                        nc.scalar.activation(t1[:], mid[:], AF.Ln)
                    # stage t = (L + a2) * b
                    for gi, mid, a2, bt in members:
                        t1, t2 = scratch[gi]
                        nc.vector.tensor_add(t1[:], t1[:], a2[:])
                        nc.vector.tensor_mul(t1[:], t1[:], bt[:])
                    # stage Square/Exp (same set as Ln): E = (2/sqrt(pi))e^{-t^2}
                    for gi, mid, a2, bt in members:
                        t1, t2 = scratch[gi]
                        nc.scalar.activation(t2[:], t1[:], AF.Square)
                        nc.scalar.activation(t2[:], t2[:], AF.Exp, scale=-1.0,
                                             bias=exp_bias)
                    # stage Erf (sigmoid set — one swap; swap back next iter)
                    for gi, mid, a2, bt in members:
                        t1, t2 = scratch[gi]
                        nc.scalar.activation(t1[:], t1[:], AF.Erf)
                    # stage decide + update
                    for gi, mid, a2, bt in members:
                        t1, t2 = scratch[gi]
                        # u = b * E
                        nc.vector.tensor_mul(t2[:], t2[:], bt[:])
                        # cmp1 = (mid - 1) * u      ( = -(1-d) b E )
                        nc.vector.scalar_tensor_tensor(
                            t2[:], mid[:], 1.0, t2[:],
                            op0=ALU.subtract, op1=ALU.mult,
                        )
                        # cmp2 = (erf + 1) * mid    ( = (1+erf) d )
                        nc.vector.scalar_tensor_tensor(
                            t1[:], t1[:], 1.0, mid[:],
                            op0=ALU.add, op1=ALU.mult,
                        )
                        # g = cmp1 + cmp2 ;  g <= 0  -> step right
                        nc.vector.tensor_add(t1[:], t1[:], t2[:])
                        # mid = (mid - w) + (g <= 0) * 2w    [fused custom op]
                        nc.vector._custom_dve(
                            step_op, out=mid[:], in0=t1[:], in1=mid[:],
                            s0=0.0, s1=w, imm2=2.0 * w,
                        )

                # ---- finish: store ----
                for gi, mid, a2, bt in members:
                    nc.sync.dma_start(out_v[gi], mid[:])

    nc.finalize()
    return nc


_CACHED = {}


def _get_nc(n_per_core, fd, group_size=2):
    key = (n_per_core, fd, group_size)
    if key not in _CACHED:
        _CACHED[key] = _build_nc(n_per_core, fd, group_size)
    return _CACHED[key]


def kernel(params: np.ndarray, bid_prices: np.ndarray) -> np.ndarray:
    from concourse.bass_utils import run_bass_kernel_spmd

    params = np.ascontiguousarray(params, dtype=np.float32)
    bid_prices = np.ascontiguousarray(bid_prices, dtype=np.float32)
    n = bid_prices.shape[0]
    n_per_core = n // N_CORES

    nc = _get_nc(n_per_core, FD)

    in_maps = []
    for i in range(N_CORES):
        sl = slice(i * n_per_core, (i + 1) * n_per_core)
        in_maps.append({"params": params[sl], "bids": bid_prices[sl]})

    res = run_bass_kernel_spmd(nc, in_maps, core_ids=list(range(N_CORES)))
    return np.concatenate([r["out"] for r in res.results], axis=0)


if __name__ == "__main__":
    # smoke test with random data
    rng = np.random.RandomState(0)
    n = N_TOTAL
    params = np.stack(
        [rng.randn(n).astype(np.float32),
         rng.uniform(0.2, 1.5, n).astype(np.float32)], axis=-1
    )
    bids = rng.uniform(0.1, 10.0, n).astype(np.float32)
    out = kernel(params=params, bid_prices=bids)
    print("out", out.shape, out.dtype, out[:8])
